# revision 16
# baseline (speedup 1.0000x reference)
"""Trainium2 Bass kernel for nn_MultiHeadAttention_45672682226228.

The reference module computes multi-head attention but everything except the
V projection is dead code (DCE'd under jit): the returned value is

    out[b, s, 64*h + q] = x[b, s, 768 + 64*h + q]
                        + sum_d x[b, s, 256*h + d] * W_v[q, d]

i.e. a per-token block-diagonal matmul (4 heads x [256 -> 64]) plus a
residual add of the last head's input slice.  W_q / W_k are unused.

Sharding: data-parallel over batch B=16 -> 2 batches (8192 tokens) per core
across 8 NeuronCores.  Inputs are shipped as bf16 (the 2e-2 rel-err budget
dwarfs bf16's ~4e-3 rounding), halving HBM read traffic, and pre-transposed
per shard so the contraction dim lands on SBUF partitions without any
on-chip transpose work.  The DRAM image IS the sequence of SBUF tiles
(xt[g, p, j, t] = x[512g + t, 128j + p]) so every partition line of a group
DMA is one contiguous 8 KiB run - big descriptors, line-rate HBM.  Per core:

  xt [16, 128, 8, 512] bf16  ->  out [16, 128, 2, 512] fp32
  (out[g, c, cc, t] = result[512g + t, 128cc + c]; host inverts both
  permutations during shard/gather)

On-chip dataflow per 512-token group (16 groups):
  1. One 1 MiB DMA loads the group tile xT [128d, 8j, 512t] bf16,
     alternating the two HWDGE rings.
  2. TensorE: outT[c-chunk, t] += Wblk_j.T @ xT_j, 4 accumulating bf16
     matmuls (N=512) per 128-wide c-chunk.  A burst of dummy warmup
     matmuls runs during the initial DMA fill so the PE_HAM clock gate
     opens (1.2 -> 2.4 GHz) before real work arrives, and per-group tile
     deps keep the PE stream dense enough that it never re-throttles.
  3. DVE adds the residual from xT chunks 6/7 (= x[:, 768:1024].T, already
     on-chip) into the PSUM result, writing fp32 outT to SBUF.
  4. SWDGE DMA stores [128, 2, 512] result tiles (4 KiB partition lines).
"""

import os
import numpy as np

P = 128
TPC = 8192          # tokens per core
NCORES = 8
GT = 512            # tokens per group
GROUPS = TPC // GT  # 16

_STATE = {}


def _pack_wblk(W_v: np.ndarray) -> np.ndarray:
    """Pack W_v [64, 256] into per-d-chunk stationary blocks [128, 8, 128].

    wblk[dd, j, col]: d-chunk j covers global d in [128j, 128j+128);
    head h = j//2, half = j%2.  Within c-chunk cc = j//4 the head's 64
    output cols sit at offset 64*(h%2).  Zeros elsewhere.
    """
    W_v = np.asarray(W_v, np.float32)
    wblk = np.zeros((P, 8, P), np.float32)
    for j in range(8):
        h, half = j // 2, j % 2
        c0 = 64 * (h % 2)
        wblk[:, j, c0:c0 + 64] = W_v[:, 128 * half:128 * half + 128].T
    return wblk


def _build_nc(tpc=TPC):
    from contextlib import ExitStack

    import concourse.mybir as mybir
    import concourse.tile as tile
    from concourse import bacc

    f32 = mybir.dt.float32
    bf16 = mybir.dt.bfloat16
    groups = tpc // GT

    nc = bacc.Bacc("TRN2", target_bir_lowering=False, debug=False)
    xt_h = nc.dram_tensor("xt", [groups, P, 8, GT], bf16, kind="ExternalInput")
    w_h = nc.dram_tensor("wblk", [P, 8, P], bf16, kind="ExternalInput")
    o_h = nc.dram_tensor("out", [groups, P, 2, GT], f32, kind="ExternalOutput")
    warm_h = nc.dram_tensor("warm", [P, 8], f32, kind="ExternalOutput")

    WARMUP = 14

    with ExitStack() as ctx:
        tc = ctx.enter_context(tile.TileContext(nc))
        const = ctx.enter_context(tc.tile_pool(name="const", bufs=1))
        xtp = ctx.enter_context(tc.tile_pool(name="xtp", bufs=8))
        osb = ctx.enter_context(tc.tile_pool(name="osb", bufs=4))
        ps_mm = ctx.enter_context(tc.tile_pool(name="ps_mm", bufs=3, space="PSUM"))
        ps_w = ctx.enter_context(tc.tile_pool(name="ps_w", bufs=1, space="PSUM"))

        w_sb = const.tile([P, 8, P], bf16)
        nc.sync.dma_start(w_sb[:], w_h[:])

        # dummy matmuls to open the PE_HAM clock gate while the input
        # stream fills; the tiny "warm" output keeps them out of DCE
        warm_ps = ps_w.tile([P, GT], f32)
        for k in range(WARMUP):
            nc.tensor.matmul(
                warm_ps[:],
                w_sb[:, 0, :],
                w_sb[:].rearrange("p j c -> p (j c)")[:, :GT],
                start=True,
                stop=True,
            )
        warm_sb = const.tile([P, 8], f32)
        nc.vector.tensor_copy(warm_sb[:], warm_ps[:, :8])
        nc.gpsimd.dma_start(warm_h[:], warm_sb[:])

        for g in range(groups):
            xt_sb = xtp.tile([P, 8, GT], bf16)
            # alternate the two HWDGE rings (SP / ACT)
            eng = nc.sync if g % 2 == 0 else nc.scalar
            eng.dma_start(xt_sb[:], xt_h[g])

            pm = ps_mm.tile([P, 2, GT], f32)
            o_sb = osb.tile([P, 2, GT], f32)
            for cc in range(2):
                for i, j in enumerate(range(4 * cc, 4 * cc + 4)):
                    nc.tensor.matmul(
                        pm[:, cc, :],
                        w_sb[:, j, :],
                        xt_sb[:, j, :],
                        start=(i == 0),
                        stop=(i == 3),
                    )
                # residual: x[:, 768:1024].T lives in xT chunks 6/7
                nc.vector.tensor_add(o_sb[:, cc, :], pm[:, cc, :], xt_sb[:, 6 + cc, :])

            # SWDGE (GpSimd) so output stores don't head-of-line block the
            # input loads on the HWDGE rings
            nc.gpsimd.dma_start(o_h[g], o_sb[:])

    nc.compile()
    return nc


def _install_ntff_hook():
    """Provide antenv.axon_hooks (absent in this image) so trace=True works.

    Reconstructs the hook trn_boot would have registered at agent boot.
    """
    import sys
    import types

    if "antenv.axon_hooks" in sys.modules:
        return
    try:
        import trn_agent_boot.trn_boot as tb

        hook = tb._ntff_profile_via_ctypes("/opt/axon/libaxon_pjrt.so")
    except Exception:
        hook = None
    mod = types.ModuleType("antenv.axon_hooks")
    mod.get_axon_ntff_profile_hook = lambda: hook
    mod.set_axon_ntff_profile_hook = lambda h: None
    sys.modules["antenv.axon_hooks"] = mod
    try:
        import antenv

        antenv.axon_hooks = mod
    except ImportError:
        pass


def kernel(x, W_q=None, W_k=None, W_v=None, **_):
    import ml_dtypes

    from concourse.bass_utils import run_bass_kernel_spmd

    if "nc" not in _STATE:
        _STATE["nc"] = _build_nc()
    nc = _STATE["nc"]

    x = np.asarray(x, np.float32)
    b, s, e = x.shape
    # xb[c, g, t, j, p] = x[core c's token 512g+t, 128j+p], cast to bf16
    xb = x.reshape(NCORES, GROUPS, GT, 8, P).astype(ml_dtypes.bfloat16)
    wblk = _pack_wblk(W_v).astype(ml_dtypes.bfloat16)

    in_maps = [
        # -> xt[g, p, j, t]
        {"xt": np.ascontiguousarray(xb[c].transpose(0, 3, 2, 1)), "wblk": wblk}
        for c in range(NCORES)
    ]
    trace = os.environ.get("KERNEL_TRACE", "0") == "1"
    if trace:
        _install_ntff_hook()
    res = run_bass_kernel_spmd(nc, in_maps, core_ids=list(range(NCORES)), trace=trace)
    _STATE["last_results"] = res
    out = np.empty((NCORES, GROUPS, GT, 2, P), np.float32)
    for c in range(NCORES):
        # out[g, c', cc, t] -> [g, t, cc, c']
        np.copyto(out[c], res.results[c]["out"].transpose(0, 3, 2, 1))
    return out.reshape(b, s, 256)


# revision 17
# speedup vs baseline: 1.1457x; 1.1457x over previous
"""Trainium2 Bass kernel for nn_MultiHeadAttention_45672682226228.

The reference module computes multi-head attention but everything except the
V projection is dead code (DCE'd under jit): the returned value is

    out[b, s, 64*h + q] = x[b, s, 768 + 64*h + q]
                        + sum_d x[b, s, 256*h + d] * W_v[q, d]

i.e. a per-token block-diagonal matmul (4 heads x [256 -> 64]) plus a
residual add of the last head's input slice.  W_q / W_k are unused.

Sharding: data-parallel over batch B=16 -> 2 batches (8192 tokens) per core
across 8 NeuronCores.  Inputs are shipped as bf16 (the 2e-2 rel-err budget
dwarfs bf16's ~4e-3 rounding), halving HBM read traffic, and pre-transposed
per shard so the contraction dim lands on SBUF partitions without any
on-chip transpose work.  The DRAM image IS the sequence of SBUF tiles
(xt[g, p, j, t] = x[512g + t, 128j + p]) so every partition line of a group
DMA is one contiguous 8 KiB run - big descriptors, line-rate HBM.  Per core:

  xt [16, 128, 8, 512] bf16  ->  out [16, 128, 2, 512] fp32
  (out[g, c, cc, t] = result[512g + t, 128cc + c]; host inverts both
  permutations during shard/gather)

On-chip dataflow per 512-token group (16 groups):
  1. One 1 MiB DMA loads the group tile xT [128d, 8j, 512t] bf16,
     alternating the two HWDGE rings.
  2. TensorE: outT[c-chunk, t] += Wblk_j.T @ xT_j, 4 accumulating bf16
     matmuls (N=512) per 128-wide c-chunk.  A burst of dummy warmup
     matmuls runs during the initial DMA fill so the PE_HAM clock gate
     opens (1.2 -> 2.4 GHz) before real work arrives, and per-group tile
     deps keep the PE stream dense enough that it never re-throttles.
  3. DVE adds the residual from xT chunks 6/7 (= x[:, 768:1024].T, already
     on-chip) into the PSUM result, writing bf16 outT to SBUF (the output
     rides back to the host as bf16 too - it upcasts during the gather).
  4. SWDGE DMA stores [128, 2, 512] result tiles (2 KiB partition lines).
"""

import os
import numpy as np

P = 128
TPC = 8192          # tokens per core
NCORES = 8
GT = 512            # tokens per group
GROUPS = TPC // GT  # 16

_STATE = {}


def _pack_wblk(W_v: np.ndarray) -> np.ndarray:
    """Pack W_v [64, 256] into per-d-chunk stationary blocks [128, 8, 128].

    wblk[dd, j, col]: d-chunk j covers global d in [128j, 128j+128);
    head h = j//2, half = j%2.  Within c-chunk cc = j//4 the head's 64
    output cols sit at offset 64*(h%2).  Zeros elsewhere.
    """
    W_v = np.asarray(W_v, np.float32)
    wblk = np.zeros((P, 8, P), np.float32)
    for j in range(8):
        h, half = j // 2, j % 2
        c0 = 64 * (h % 2)
        wblk[:, j, c0:c0 + 64] = W_v[:, 128 * half:128 * half + 128].T
    return wblk


def _build_nc(tpc=TPC):
    from contextlib import ExitStack

    import concourse.mybir as mybir
    import concourse.tile as tile
    from concourse import bacc

    f32 = mybir.dt.float32
    bf16 = mybir.dt.bfloat16
    groups = tpc // GT

    nc = bacc.Bacc("TRN2", target_bir_lowering=False, debug=False)
    xt_h = nc.dram_tensor("xt", [groups, P, 8, GT], bf16, kind="ExternalInput")
    w_h = nc.dram_tensor("wblk", [P, 8, P], bf16, kind="ExternalInput")
    o_h = nc.dram_tensor("out", [groups, P, 2, GT], bf16, kind="ExternalOutput")
    warm_h = nc.dram_tensor("warm", [P, 8], f32, kind="ExternalOutput")

    WARMUP = 14

    with ExitStack() as ctx:
        tc = ctx.enter_context(tile.TileContext(nc))
        const = ctx.enter_context(tc.tile_pool(name="const", bufs=1))
        xtp = ctx.enter_context(tc.tile_pool(name="xtp", bufs=12))
        osb = ctx.enter_context(tc.tile_pool(name="osb", bufs=6))
        ps_mm = ctx.enter_context(tc.tile_pool(name="ps_mm", bufs=3, space="PSUM"))
        ps_w = ctx.enter_context(tc.tile_pool(name="ps_w", bufs=1, space="PSUM"))

        w_sb = const.tile([P, 8, P], bf16)
        nc.sync.dma_start(w_sb[:], w_h[:])

        # dummy matmuls to open the PE_HAM clock gate while the input
        # stream fills; the tiny "warm" output keeps them out of DCE
        warm_ps = ps_w.tile([P, GT], f32)
        for k in range(WARMUP):
            nc.tensor.matmul(
                warm_ps[:],
                w_sb[:, 0, :],
                w_sb[:].rearrange("p j c -> p (j c)")[:, :GT],
                start=True,
                stop=True,
            )
        warm_sb = const.tile([P, 8], f32)
        nc.vector.tensor_copy(warm_sb[:], warm_ps[:, :8])
        nc.gpsimd.dma_start(warm_h[:], warm_sb[:])

        for g in range(groups):
            xt_sb = xtp.tile([P, 8, GT], bf16)
            # alternate the two HWDGE rings (SP / ACT)
            eng = nc.sync if g % 2 == 0 else nc.scalar
            eng.dma_start(xt_sb[:], xt_h[g])

            pm = ps_mm.tile([P, 2, GT], f32)
            o_sb = osb.tile([P, 2, GT], bf16)
            for cc in range(2):
                for i, j in enumerate(range(4 * cc, 4 * cc + 4)):
                    nc.tensor.matmul(
                        pm[:, cc, :],
                        w_sb[:, j, :],
                        xt_sb[:, j, :],
                        start=(i == 0),
                        stop=(i == 3),
                    )
                # residual: x[:, 768:1024].T lives in xT chunks 6/7
                nc.vector.tensor_add(o_sb[:, cc, :], pm[:, cc, :], xt_sb[:, 6 + cc, :])

            # SWDGE (GpSimd) so output stores don't head-of-line block the
            # input loads on the HWDGE rings
            nc.gpsimd.dma_start(o_h[g], o_sb[:])

    nc.compile()
    return nc


def _install_ntff_hook():
    """Provide antenv.axon_hooks (absent in this image) so trace=True works.

    Reconstructs the hook trn_boot would have registered at agent boot.
    """
    import sys
    import types

    if "antenv.axon_hooks" in sys.modules:
        return
    try:
        import trn_agent_boot.trn_boot as tb

        hook = tb._ntff_profile_via_ctypes("/opt/axon/libaxon_pjrt.so")
    except Exception:
        hook = None
    mod = types.ModuleType("antenv.axon_hooks")
    mod.get_axon_ntff_profile_hook = lambda: hook
    mod.set_axon_ntff_profile_hook = lambda h: None
    sys.modules["antenv.axon_hooks"] = mod
    try:
        import antenv

        antenv.axon_hooks = mod
    except ImportError:
        pass


def kernel(x, W_q=None, W_k=None, W_v=None, **_):
    import ml_dtypes

    from concourse.bass_utils import run_bass_kernel_spmd

    if "nc" not in _STATE:
        _STATE["nc"] = _build_nc()
    nc = _STATE["nc"]

    x = np.asarray(x, np.float32)
    b, s, e = x.shape
    # xb[c, g, t, j, p] = x[core c's token 512g+t, 128j+p], cast to bf16
    xb = x.reshape(NCORES, GROUPS, GT, 8, P).astype(ml_dtypes.bfloat16)
    wblk = _pack_wblk(W_v).astype(ml_dtypes.bfloat16)

    in_maps = [
        # -> xt[g, p, j, t]
        {"xt": np.ascontiguousarray(xb[c].transpose(0, 3, 2, 1)), "wblk": wblk}
        for c in range(NCORES)
    ]
    trace = os.environ.get("KERNEL_TRACE", "0") == "1"
    if trace:
        _install_ntff_hook()
    res = run_bass_kernel_spmd(nc, in_maps, core_ids=list(range(NCORES)), trace=trace)
    _STATE["last_results"] = res
    out = np.empty((NCORES, GROUPS, GT, 2, P), np.float32)
    for c in range(NCORES):
        # out[g, c', cc, t] -> [g, t, cc, c']
        np.copyto(out[c], res.results[c]["out"].transpose(0, 3, 2, 1))
    return out.reshape(b, s, 256)


# revision 19
# speedup vs baseline: 1.1616x; 1.0139x over previous
"""Trainium2 Bass kernel for nn_MultiHeadAttention_45672682226228.

The reference module computes multi-head attention but everything except the
V projection is dead code (DCE'd under jit): the returned value is

    out[b, s, 64*h + q] = x[b, s, 768 + 64*h + q]
                        + sum_d x[b, s, 256*h + d] * W_v[q, d]

i.e. a per-token block-diagonal matmul (4 heads x [256 -> 64]) plus a
residual add of the last head's input slice.  W_q / W_k are unused.

Sharding: data-parallel over batch B=16 -> 2 batches (8192 tokens) per core
across 8 NeuronCores.  Inputs are shipped as bf16 (the 2e-2 rel-err budget
dwarfs bf16's ~4e-3 rounding), halving HBM read traffic, and pre-transposed
per shard so the contraction dim lands on SBUF partitions without any
on-chip transpose work.  The DRAM image IS the sequence of SBUF tiles
(xt[g, p, j, t] = x[512g + t, 128j + p]) so every partition line of a group
DMA is one contiguous 8 KiB run - big descriptors, line-rate HBM.  Per core:

  xt [16, 128, 8, 512] bf16  ->  out [16, 128, 2, 512] fp32
  (out[g, c, cc, t] = result[512g + t, 128cc + c]; host inverts both
  permutations during shard/gather)

On-chip dataflow per 512-token group (16 groups):
  1. All 16 group tiles xT [128d, 8j, 512t] bf16 are statically resident
     (128 KiB/partition); all 16 input DMAs issue up front, alternating the
     two HWDGE rings, so the input stream runs back-to-back at line rate
     with nothing gating it.
  2. TensorE: outT[c-chunk, t] += Wblk_j.T @ xT_j, 4 accumulating bf16
     matmuls (N=512) per 128-wide c-chunk.  A burst of dummy warmup
     matmuls runs during the initial DMA fill so the PE_HAM clock gate
     opens (1.2 -> 2.4 GHz) before real work arrives, and per-group tile
     deps keep the PE stream dense enough that it never re-throttles.
  3. DVE adds the residual from xT chunks 6/7 (= x[:, 768:1024].T, already
     on-chip) into the PSUM result, writing bf16 outT to SBUF (the output
     rides back to the host as bf16 too - it upcasts during the gather).
  4. SWDGE DMA stores [128, 2, 512] result tiles (2 KiB partition lines).
"""

import os
import numpy as np

P = 128
TPC = 8192          # tokens per core
NCORES = 8
GT = 512            # tokens per group
GROUPS = TPC // GT  # 16

_STATE = {}


def _pack_wblk(W_v: np.ndarray) -> np.ndarray:
    """Pack W_v [64, 256] into per-d-chunk stationary blocks [128, 8, 128].

    wblk[dd, j, col]: d-chunk j covers global d in [128j, 128j+128);
    head h = j//2, half = j%2.  Within c-chunk cc = j//4 the head's 64
    output cols sit at offset 64*(h%2).  Zeros elsewhere.
    """
    W_v = np.asarray(W_v, np.float32)
    wblk = np.zeros((P, 8, P), np.float32)
    for j in range(8):
        h, half = j // 2, j % 2
        c0 = 64 * (h % 2)
        wblk[:, j, c0:c0 + 64] = W_v[:, 128 * half:128 * half + 128].T
    return wblk


def _build_nc(tpc=TPC):
    from contextlib import ExitStack

    import concourse.mybir as mybir
    import concourse.tile as tile
    from concourse import bacc

    f32 = mybir.dt.float32
    bf16 = mybir.dt.bfloat16
    groups = tpc // GT

    nc = bacc.Bacc("TRN2", target_bir_lowering=False, debug=False)
    xt_h = nc.dram_tensor("xt", [groups, P, 8, GT], bf16, kind="ExternalInput")
    w_h = nc.dram_tensor("wblk", [P, 8, P], bf16, kind="ExternalInput")
    o_h = nc.dram_tensor("out", [groups, P, 2, GT], bf16, kind="ExternalOutput")
    warm_h = nc.dram_tensor("warm", [P, 8], f32, kind="ExternalOutput")

    WARMUP = 14

    with ExitStack() as ctx:
        tc = ctx.enter_context(tile.TileContext(nc))
        const = ctx.enter_context(tc.tile_pool(name="const", bufs=1))
        xtp = ctx.enter_context(tc.tile_pool(name="xtp", bufs=1))
        osb = ctx.enter_context(tc.tile_pool(name="osb", bufs=8))
        ps_mm = ctx.enter_context(tc.tile_pool(name="ps_mm", bufs=3, space="PSUM"))
        ps_w = ctx.enter_context(tc.tile_pool(name="ps_w", bufs=1, space="PSUM"))

        w_sb = const.tile([P, 8, P], bf16)
        nc.sync.dma_start(w_sb[:], w_h[:])

        # every group tile is statically resident; issue the whole input
        # stream immediately on the two HWDGE rings
        xts = [xtp.tile([P, 8, GT], bf16, name=f"xt{g}") for g in range(groups)]
        for g in range(groups):
            eng = nc.sync if g % 2 == 0 else nc.scalar
            eng.dma_start(xts[g][:], xt_h[g])

        # dummy matmuls to open the PE_HAM clock gate while the input
        # stream fills; the tiny "warm" output keeps them out of DCE
        warm_ps = ps_w.tile([P, GT], f32)
        for k in range(WARMUP):
            nc.tensor.matmul(
                warm_ps[:],
                w_sb[:, 0, :],
                w_sb[:].rearrange("p j c -> p (j c)")[:, :GT],
                start=True,
                stop=True,
            )
        warm_sb = const.tile([P, 8], f32)
        nc.vector.tensor_copy(warm_sb[:], warm_ps[:, :8])
        nc.gpsimd.dma_start(warm_h[:], warm_sb[:])

        for g in range(groups):
            xt_sb = xts[g]
            pm = ps_mm.tile([P, 2, GT], f32)
            o_sb = osb.tile([P, 2, GT], bf16)
            for cc in range(2):
                for i, j in enumerate(range(4 * cc, 4 * cc + 4)):
                    nc.tensor.matmul(
                        pm[:, cc, :],
                        w_sb[:, j, :],
                        xt_sb[:, j, :],
                        start=(i == 0),
                        stop=(i == 3),
                    )
                # residual: x[:, 768:1024].T lives in xT chunks 6/7
                nc.vector.tensor_add(o_sb[:, cc, :], pm[:, cc, :], xt_sb[:, 6 + cc, :])

            # SWDGE (GpSimd) so output stores don't head-of-line block the
            # input loads on the HWDGE rings
            nc.gpsimd.dma_start(o_h[g], o_sb[:])

    nc.compile()
    return nc


def _install_ntff_hook():
    """Provide antenv.axon_hooks (absent in this image) so trace=True works.

    Reconstructs the hook trn_boot would have registered at agent boot.
    """
    import sys
    import types

    if "antenv.axon_hooks" in sys.modules:
        return
    try:
        import trn_agent_boot.trn_boot as tb

        hook = tb._ntff_profile_via_ctypes("/opt/axon/libaxon_pjrt.so")
    except Exception:
        hook = None
    mod = types.ModuleType("antenv.axon_hooks")
    mod.get_axon_ntff_profile_hook = lambda: hook
    mod.set_axon_ntff_profile_hook = lambda h: None
    sys.modules["antenv.axon_hooks"] = mod
    try:
        import antenv

        antenv.axon_hooks = mod
    except ImportError:
        pass


def kernel(x, W_q=None, W_k=None, W_v=None, **_):
    import ml_dtypes

    from concourse.bass_utils import run_bass_kernel_spmd

    if "nc" not in _STATE:
        _STATE["nc"] = _build_nc()
    nc = _STATE["nc"]

    x = np.asarray(x, np.float32)
    b, s, e = x.shape
    # xb[c, g, t, j, p] = x[core c's token 512g+t, 128j+p], cast to bf16
    xb = x.reshape(NCORES, GROUPS, GT, 8, P).astype(ml_dtypes.bfloat16)
    wblk = _pack_wblk(W_v).astype(ml_dtypes.bfloat16)

    in_maps = [
        # -> xt[g, p, j, t]
        {"xt": np.ascontiguousarray(xb[c].transpose(0, 3, 2, 1)), "wblk": wblk}
        for c in range(NCORES)
    ]
    trace = os.environ.get("KERNEL_TRACE", "0") == "1"
    if trace:
        _install_ntff_hook()
    res = run_bass_kernel_spmd(nc, in_maps, core_ids=list(range(NCORES)), trace=trace)
    _STATE["last_results"] = res
    out = np.empty((NCORES, GROUPS, GT, 2, P), np.float32)
    for c in range(NCORES):
        # out[g, c', cc, t] -> [g, t, cc, c']
        np.copyto(out[c], res.results[c]["out"].transpose(0, 3, 2, 1))
    return out.reshape(b, s, 256)


# revision 20
# speedup vs baseline: 1.2744x; 1.0971x over previous
"""Trainium2 Bass kernel for nn_MultiHeadAttention_45672682226228.

The reference module computes multi-head attention but everything except the
V projection is dead code (DCE'd under jit): the returned value is

    out[b, s, 64*h + q] = x[b, s, 768 + 64*h + q]
                        + sum_d x[b, s, 256*h + d] * W_v[q, d]

i.e. a per-token block-diagonal matmul (4 heads x [256 -> 64]) plus a
residual add of the last head's input slice.  W_q / W_k are unused.

Sharding: data-parallel over batch B=16 -> 2 batches (8192 tokens) per core
across 8 NeuronCores.  Inputs are shipped as bf16 (the 2e-2 rel-err budget
dwarfs bf16's ~4e-3 rounding), halving HBM read traffic, and pre-transposed
per shard so the contraction dim lands on SBUF partitions without any
on-chip transpose work.  The DRAM image IS the sequence of SBUF tiles
(xt[g, p, j, t] = x[512g + t, 128j + p]) so every partition line of a group
DMA is one contiguous 8 KiB run - big descriptors, line-rate HBM.  Per core:

  xt [8, 128, 2, 8, 512] bf16  ->  out [16, 128, 2, 512] bf16
  (out[g, c, cc, t] = result[512g + t, 128cc + c]; host inverts both
  permutations during shard/gather)

On-chip dataflow per 512-token group (16 groups):
  1. All input tiles are statically resident (128 KiB/partition) as 8
     two-group blocks [128d, 2g, 8j, 512t] bf16; all 8 block DMAs (2 MiB,
     16 KiB contiguous partition lines) issue up front on the two HWDGE
     rings.  Big transfers amortize the ~2.5 us per-DMA completion receipt
     that otherwise pokes holes in the stream (8 in-flight sem lanes).
  2. TensorE: outT[c-chunk, t] += Wblk_j.T @ xT_j, 4 accumulating bf16
     matmuls (N=512) per 128-wide c-chunk.  A burst of dummy warmup
     matmuls runs during the initial DMA fill so the PE_HAM clock gate
     opens (1.2 -> 2.4 GHz) before real work arrives, and per-group tile
     deps keep the PE stream dense enough that it never re-throttles.
  3. DVE adds the residual from xT chunks 6/7 (= x[:, 768:1024].T, already
     on-chip) into the PSUM result, writing bf16 outT to SBUF (the output
     rides back to the host as bf16 too - it upcasts during the gather).
  4. SWDGE DMA stores [128, 2, 512] result tiles (2 KiB partition lines).
"""

import os
import numpy as np

P = 128
TPC = 8192          # tokens per core
NCORES = 8
GT = 512            # tokens per group
GROUPS = TPC // GT  # 16

_STATE = {}


def _pack_wblk(W_v: np.ndarray) -> np.ndarray:
    """Pack W_v [64, 256] into per-d-chunk stationary blocks [128, 8, 128].

    wblk[dd, j, col]: d-chunk j covers global d in [128j, 128j+128);
    head h = j//2, half = j%2.  Within c-chunk cc = j//4 the head's 64
    output cols sit at offset 64*(h%2).  Zeros elsewhere.
    """
    W_v = np.asarray(W_v, np.float32)
    wblk = np.zeros((P, 8, P), np.float32)
    for j in range(8):
        h, half = j // 2, j % 2
        c0 = 64 * (h % 2)
        wblk[:, j, c0:c0 + 64] = W_v[:, 128 * half:128 * half + 128].T
    return wblk


def _build_nc(tpc=TPC):
    from contextlib import ExitStack

    import concourse.mybir as mybir
    import concourse.tile as tile
    from concourse import bacc

    f32 = mybir.dt.float32
    bf16 = mybir.dt.bfloat16
    groups = tpc // GT

    nc = bacc.Bacc("TRN2", target_bir_lowering=False, debug=False)
    xt_h = nc.dram_tensor("xt", [groups // 2, P, 2, 8, GT], bf16, kind="ExternalInput")
    w_h = nc.dram_tensor("wblk", [P, 8, P], bf16, kind="ExternalInput")
    o_h = nc.dram_tensor("out", [groups, P, 2, GT], bf16, kind="ExternalOutput")
    warm_h = nc.dram_tensor("warm", [P, 8], f32, kind="ExternalOutput")

    WARMUP = 40

    with ExitStack() as ctx:
        tc = ctx.enter_context(tile.TileContext(nc))
        const = ctx.enter_context(tc.tile_pool(name="const", bufs=1))
        xtp = ctx.enter_context(tc.tile_pool(name="xtp", bufs=1))
        osb = ctx.enter_context(tc.tile_pool(name="osb", bufs=8))
        ps_mm = ctx.enter_context(tc.tile_pool(name="ps_mm", bufs=3, space="PSUM"))
        ps_w = ctx.enter_context(tc.tile_pool(name="ps_w", bufs=1, space="PSUM"))

        w_sb = const.tile([P, 8, P], bf16)
        nc.sync.dma_start(w_sb[:], w_h[:])

        # every input block is statically resident; issue the whole input
        # stream immediately on the two HWDGE rings
        blocks = groups // 2
        xts = [xtp.tile([P, 2, 8, GT], bf16, name=f"xt{b}") for b in range(blocks)]
        for b in range(blocks):
            eng = nc.sync if b % 2 == 0 else nc.scalar
            eng.dma_start(xts[b][:], xt_h[b])

        # dummy matmuls to open the PE_HAM clock gate while the input
        # stream fills; the tiny "warm" output keeps them out of DCE
        warm_ps = ps_w.tile([P, GT], f32)
        for k in range(WARMUP):
            nc.tensor.matmul(
                warm_ps[:],
                w_sb[:, 0, :],
                w_sb[:].rearrange("p j c -> p (j c)")[:, :GT],
                start=True,
                stop=True,
            )
        warm_sb = const.tile([P, 8], f32)
        nc.vector.tensor_copy(warm_sb[:], warm_ps[:, :8])
        nc.gpsimd.dma_start(warm_h[:], warm_sb[:])

        for g in range(groups):
            xt_sb = xts[g // 2][:, g % 2]
            pm = ps_mm.tile([P, 2, GT], f32)
            o_sb = osb.tile([P, 2, GT], bf16)
            for cc in range(2):
                for i, j in enumerate(range(4 * cc, 4 * cc + 4)):
                    nc.tensor.matmul(
                        pm[:, cc, :],
                        w_sb[:, j, :],
                        xt_sb[:, j, :],
                        start=(i == 0),
                        stop=(i == 3),
                    )
                # residual: x[:, 768:1024].T lives in xT chunks 6/7
                nc.vector.tensor_add(o_sb[:, cc, :], pm[:, cc, :], xt_sb[:, 6 + cc, :])

            # SWDGE (GpSimd) so output stores don't head-of-line block the
            # input loads on the HWDGE rings
            nc.gpsimd.dma_start(o_h[g], o_sb[:])

    nc.compile()
    return nc


def _install_ntff_hook():
    """Provide antenv.axon_hooks (absent in this image) so trace=True works.

    Reconstructs the hook trn_boot would have registered at agent boot.
    """
    import sys
    import types

    if "antenv.axon_hooks" in sys.modules:
        return
    try:
        import trn_agent_boot.trn_boot as tb

        hook = tb._ntff_profile_via_ctypes("/opt/axon/libaxon_pjrt.so")
    except Exception:
        hook = None
    mod = types.ModuleType("antenv.axon_hooks")
    mod.get_axon_ntff_profile_hook = lambda: hook
    mod.set_axon_ntff_profile_hook = lambda h: None
    sys.modules["antenv.axon_hooks"] = mod
    try:
        import antenv

        antenv.axon_hooks = mod
    except ImportError:
        pass


def kernel(x, W_q=None, W_k=None, W_v=None, **_):
    import ml_dtypes

    from concourse.bass_utils import run_bass_kernel_spmd

    if "nc" not in _STATE:
        _STATE["nc"] = _build_nc()
    nc = _STATE["nc"]

    x = np.asarray(x, np.float32)
    b, s, e = x.shape
    # xb[c, b, g', t, j, p] = x[core c's token 512*(2b+g')+t, 128j+p]
    xb = x.reshape(NCORES, GROUPS // 2, 2, GT, 8, P).astype(ml_dtypes.bfloat16)
    wblk = _pack_wblk(W_v).astype(ml_dtypes.bfloat16)

    in_maps = [
        # -> xt[b, p, g', j, t]
        {"xt": np.ascontiguousarray(xb[c].transpose(0, 4, 1, 3, 2)), "wblk": wblk}
        for c in range(NCORES)
    ]
    trace = os.environ.get("KERNEL_TRACE", "0") == "1"
    if trace:
        _install_ntff_hook()
    res = run_bass_kernel_spmd(nc, in_maps, core_ids=list(range(NCORES)), trace=trace)
    _STATE["last_results"] = res
    out = np.empty((NCORES, GROUPS, GT, 2, P), np.float32)
    for c in range(NCORES):
        # out[g, c', cc, t] -> [g, t, cc, c']
        np.copyto(out[c], res.results[c]["out"].transpose(0, 3, 2, 1))
    return out.reshape(b, s, 256)


# revision 25
# speedup vs baseline: 1.3123x; 1.0297x over previous
"""Trainium2 Bass kernel for nn_MultiHeadAttention_45672682226228.

The reference module computes multi-head attention but everything except the
V projection is dead code (DCE'd under jit): the returned value is

    out[b, s, 64*h + q] = x[b, s, 768 + 64*h + q]
                        + sum_d x[b, s, 256*h + d] * W_v[q, d]

i.e. a per-token block-diagonal matmul (4 heads x [256 -> 64]) plus a
residual add of the last head's input slice.  W_q / W_k are unused.

Sharding: data-parallel over batch B=16 -> 2 batches (8192 tokens) per core
across 8 NeuronCores.  Inputs are shipped as bf16 (the 2e-2 rel-err budget
dwarfs bf16's ~4e-3 rounding), halving HBM read traffic, and pre-transposed
per shard so the contraction dim lands on SBUF partitions without any
on-chip transpose work.  The DRAM image IS the sequence of SBUF tiles
(xt[g, p, j, t] = x[512g + t, 128j + p]) so every partition line of a group
DMA is one contiguous 8 KiB run - big descriptors, line-rate HBM.  Per core:

  xt [8, 128, 2, 8, 512] bf16  ->  out [16, 128, 2, 512] bf16
  (out[g, c, cc, t] = result[512g + t, 128cc + c]; host inverts both
  permutations during shard/gather)

On-chip dataflow per 512-token group (16 groups):
  1. All input tiles are statically resident (128 KiB/partition) as 8
     two-group blocks [128d, 2g, 8j, 512t] bf16; all 8 block DMAs (2 MiB,
     16 KiB contiguous partition lines) issue up front on the two HWDGE
     rings.  Big transfers amortize the ~2.5 us per-DMA completion receipt
     that otherwise pokes holes in the stream (8 in-flight sem lanes).
  2. TensorE: outT[c-chunk, t] += Wblk_j.T @ xT_j, 4 accumulating bf16
     matmuls (N=512) per 128-wide c-chunk.  j-outer over the block's two
     groups: the second matmul of each pair sets ldweights=False and
     reuses the W_j already in the array, halving LDWEIGHTS traffic.  A
     burst of dummy warmup matmuls runs during the initial DMA fill so the
     PE_HAM clock gate opens (1.2 -> 2.4 GHz) before real work arrives.
  3. DVE adds the residual from xT chunks 6/7 (= x[:, 768:1024].T, already
     on-chip) into the PSUM result, writing bf16 outT to SBUF (the output
     rides back to the host as bf16 too - it upcasts during the gather).
  4. SWDGE DMA stores [128, 2, 512] result tiles (2 KiB partition lines).
"""

import os
import numpy as np

P = 128
TPC = 8192          # tokens per core
NCORES = 8
GT = 512            # tokens per group
GROUPS = TPC // GT  # 16

_STATE = {}


def _pack_wblk(W_v: np.ndarray) -> np.ndarray:
    """Pack W_v [64, 256] into per-d-chunk stationary blocks [128, 8, 128].

    wblk[dd, j, col]: d-chunk j covers global d in [128j, 128j+128);
    head h = j//2, half = j%2.  Within c-chunk cc = j//4 the head's 64
    output cols sit at offset 64*(h%2).  Zeros elsewhere.
    """
    W_v = np.asarray(W_v, np.float32)
    wblk = np.zeros((P, 8, P), np.float32)
    for j in range(8):
        h, half = j // 2, j % 2
        c0 = 64 * (h % 2)
        wblk[:, j, c0:c0 + 64] = W_v[:, 128 * half:128 * half + 128].T
    return wblk


def _build_nc(tpc=TPC):
    from contextlib import ExitStack

    import concourse.mybir as mybir
    import concourse.tile as tile
    from concourse import bacc

    f32 = mybir.dt.float32
    bf16 = mybir.dt.bfloat16
    groups = tpc // GT

    nc = bacc.Bacc("TRN2", target_bir_lowering=False, debug=False)
    xt_h = nc.dram_tensor("xt", [groups // 2, P, 2, 8, GT], bf16, kind="ExternalInput")
    w_h = nc.dram_tensor("wblk", [P, 8, P], bf16, kind="ExternalInput")
    o_h = nc.dram_tensor("out", [groups, P, 2, GT], bf16, kind="ExternalOutput")
    warm_h = nc.dram_tensor("warm", [P, 8], f32, kind="ExternalOutput")

    WARMUP = 40

    with ExitStack() as ctx:
        tc = ctx.enter_context(tile.TileContext(nc))
        const = ctx.enter_context(tc.tile_pool(name="const", bufs=1))
        xtp = ctx.enter_context(tc.tile_pool(name="xtp", bufs=1))
        osb = ctx.enter_context(tc.tile_pool(name="osb", bufs=4))
        ps_mm = ctx.enter_context(tc.tile_pool(name="ps_mm", bufs=2, space="PSUM"))

        w_sb = const.tile([P, 8, P], bf16)
        nc.sync.dma_start(w_sb[:], w_h[:])

        # every input block is statically resident; issue the whole input
        # stream immediately on the two HWDGE rings
        blocks = groups // 2
        xts = [xtp.tile([P, 2, 8, GT], bf16, name=f"xt{b}") for b in range(blocks)]
        for b in range(blocks):
            eng = nc.sync if b % 2 == 0 else nc.scalar
            eng.dma_start(xts[b][:], xt_h[b])

        # dummy matmuls to open the PE_HAM clock gate while the input
        # stream fills; the tiny "warm" output keeps them out of DCE
        warm_ps = ps_mm.tile([P, 4, GT], f32, name="pm")
        for k in range(WARMUP):
            wm = nc.tensor.matmul(
                warm_ps[:, 0, :],
                w_sb[:, 0, :],
                w_sb[:].rearrange("p j c -> p (j c)")[:, :GT],
                start=True,
                stop=True,
            )
            if k > 0:
                wm.ins.ldweights = False
        warm_sb = const.tile([P, 8], f32)
        nc.vector.tensor_copy(warm_sb[:], warm_ps[:, 0, :8])
        nc.gpsimd.dma_start(warm_h[:], warm_sb[:])

        for b in range(blocks):
            # psum banks: [g' * 2 + cc]
            pm = ps_mm.tile([P, 4, GT], f32)
            o_sb = osb.tile([P, 2, 2, GT], bf16)
            for j in range(8):
                cc = j // 4
                for gp in range(2):
                    m = nc.tensor.matmul(
                        pm[:, 2 * gp + cc, :],
                        w_sb[:, j, :],
                        xts[b][:, gp, j, :],
                        start=(j % 4 == 0),
                        stop=(j % 4 == 3),
                    )
                    if gp == 1:
                        # W_j is already in the array from the gp=0 matmul
                        m.ins.ldweights = False

            for gp in range(2):
                g = 2 * b + gp
                # residual: x[:, 768:1024].T lives in xT chunks 6/7
                for cc in range(2):
                    nc.vector.tensor_add(
                        o_sb[:, gp, cc, :],
                        pm[:, 2 * gp + cc, :],
                        xts[b][:, gp, 6 + cc, :],
                    )
                if b < blocks - 2:
                    # SWDGE (GpSimd) so output stores don't head-of-line
                    # block the input loads on the HWDGE rings
                    nc.gpsimd.dma_start(o_h[g], o_sb[:, gp])
                else:
                    # input stream is done by now; the low-latency HWDGE
                    # rings are free - use them to compress the tail
                    eng = nc.sync if gp == 0 else nc.scalar
                    eng.dma_start(o_h[g], o_sb[:, gp])

    nc.compile()
    return nc


def _install_ntff_hook():
    """Provide antenv.axon_hooks (absent in this image) so trace=True works.

    Reconstructs the hook trn_boot would have registered at agent boot.
    """
    import sys
    import types

    if "antenv.axon_hooks" in sys.modules:
        return
    try:
        import trn_agent_boot.trn_boot as tb

        hook = tb._ntff_profile_via_ctypes("/opt/axon/libaxon_pjrt.so")
    except Exception:
        hook = None
    mod = types.ModuleType("antenv.axon_hooks")
    mod.get_axon_ntff_profile_hook = lambda: hook
    mod.set_axon_ntff_profile_hook = lambda h: None
    sys.modules["antenv.axon_hooks"] = mod
    try:
        import antenv

        antenv.axon_hooks = mod
    except ImportError:
        pass


def kernel(x, W_q=None, W_k=None, W_v=None, **_):
    import ml_dtypes

    from concourse.bass_utils import run_bass_kernel_spmd

    if "nc" not in _STATE:
        _STATE["nc"] = _build_nc()
    nc = _STATE["nc"]

    x = np.asarray(x, np.float32)
    b, s, e = x.shape
    # xb[c, b, g', t, j, p] = x[core c's token 512*(2b+g')+t, 128j+p]
    xb = x.reshape(NCORES, GROUPS // 2, 2, GT, 8, P).astype(ml_dtypes.bfloat16)
    wblk = _pack_wblk(W_v).astype(ml_dtypes.bfloat16)

    in_maps = [
        # -> xt[b, p, g', j, t]
        {"xt": np.ascontiguousarray(xb[c].transpose(0, 4, 1, 3, 2)), "wblk": wblk}
        for c in range(NCORES)
    ]
    trace = os.environ.get("KERNEL_TRACE", "0") == "1"
    if trace:
        _install_ntff_hook()
    res = run_bass_kernel_spmd(nc, in_maps, core_ids=list(range(NCORES)), trace=trace)
    _STATE["last_results"] = res
    out = np.empty((NCORES, GROUPS, GT, 2, P), np.float32)
    for c in range(NCORES):
        # out[g, c', cc, t] -> [g, t, cc, c']
        np.copyto(out[c], res.results[c]["out"].transpose(0, 3, 2, 1))
    return out.reshape(b, s, 256)
